# revision 11
# baseline (speedup 1.0000x reference)
"""Trainium2 Bass kernel for nn_CrossAttention_43258910605402.

Masked cross-attention, head-parallel over 8 NeuronCores (one head per core).

Math (per head h):
  q = x @ Wq[:, 64h:64h+64] * d^-0.5          [n=6912, 64]
  k = ctx @ Wk[:, 64h:64h+64]                 [m=3072, 64]
  v = ctx @ Wv[:, 64h:64h+64]                 [m=3072, 64]
  S = q @ k^T  (masked entries -> -inf)       [n, m]
  A = exp(S)   (no row-max: |S| <= ~10 here; masked -> exp = 0)
  out_h = (A @ v) / rowsum(A)                 [n, 64]
  partial = out_h @ Wo[64h:64h+64, :]         [n, 320]
Host: out = sum_h partial_h + bo.

The mask is rank-1 (m1_i & m2_j).  Host permutes q rows and k cols so
unmasked entries come first (n0 / m0 split).  Chunks of q are then PURE:
unmasked chunks attend all of k with no mask at all; masked chunks attend
only the first ceil(m0/128) k-tiles, and only the last (partial) k-tile
needs masking, applied as a per-partition bias (-1e30) on the exp
activation -- so the contraction stays exactly 64.

With K=64 the QK^T matmuls run 2-way row-tiled: even k-tiles occupy PE
rows 0-63 (stationary at SBUF partitions 0-63), odd k-tiles rows 64-127,
executing concurrently into the two halves of one [128, 1024] PSUM pair.
One exp ACTIVATE covers the pair.  A@V uses a [128, 65] stationary
(v plus a ones column that yields rowsum(A) as output row 64);
normalization is deferred into a per-partition scalar multiply after the
output projection.  All matmul operands are bf16 (fp32 PSUM accumulate),
which also halves input DMA.
"""

import numpy as np

HEADS = 8
D = 64
DA = 65          # d + 1 ones row for fused rowsum
N = 6912         # query positions
M = 3072         # key positions
C = 320          # model dim
SCALE = D ** -0.5
NEG = -1e30
NKT = M // 128

_compiled = {}
_last_in_maps = None
_last_key = None


def _chunks(total, size, base=0):
    out = []
    o = 0
    while o < total:
        w = min(size, total - o)
        out.append((base + o, w))
        o += w
    return out


def _build_program(n0=N, m0=M, QCHUNK=512):
    # n0: q rows 0..n0-1 are unmasked (attend all k); rows n0.. attend only
    # k < m0.  NKT_SHORT = ceil(m0/128) k-tiles, with a -1e30 per-partition
    # exp bias killing the masked tail of the last short tile.
    import concourse.bacc as bacc
    import concourse.tile as tile
    import concourse.mybir as mybir

    NKT_SHORT = max(1, -(-m0 // 128))
    bias_tile = NKT_SHORT - 1 if (m0 % 128 != 0 and n0 < N) else None
    f32 = mybir.dt.float32
    f32r = mybir.dt.float32r
    bf16 = mybir.dt.bfloat16
    EXP = mybir.ActivationFunctionType.Exp

    nc = bacc.Bacc("TRN2", target_bir_lowering=False, debug=False)

    xt_d = nc.dram_tensor("xt", [C, N], bf16, kind="ExternalInput").ap()
    ctxt_d = nc.dram_tensor("ctxt", [C, M], bf16, kind="ExternalInput").ap()
    # packed weights: [128, 896] bf16 = wq(192) wk(192) wv(192) | wo 64x320
    wp_d = nc.dram_tensor("wpack", [128, 896], bf16, kind="ExternalInput").ap()
    eye_d = nc.dram_tensor("eye", [64, 64], f32r, kind="ExternalInput").ap()
    m2n_d = nc.dram_tensor("m2neg", [128, NKT], f32, kind="ExternalInput").ap()
    out_d = nc.dram_tensor("out", [N, C], f32, kind="ExternalOutput").ap()

    CCH = [(0, 128), (128, 128), (256, 64)]   # contraction tiles over C=320

    with tile.TileContext(nc) as tc:
        with (
            tc.tile_pool(name="persist", bufs=1) as persist,
            tc.tile_pool(name="stage", bufs=3) as stage,
            tc.tile_pool(name="qpool", bufs=2) as qpool,
            tc.tile_pool(name="attn", bufs=3) as apool,
            tc.tile_pool(name="oc", bufs=2) as ocpool,
            tc.tile_pool(name="outsb", bufs=3) as outsb,
        ):
            # ---- constants / weights -------------------------------------
            wp_r = persist.tile([128, 896], bf16, tag="wpack")
            nc.sync.dma_start(wp_r[:], wp_d[:])
            eye = persist.tile([64, 64], f32r, tag="eye")
            nc.sync.dma_start(eye[:], eye_d[:])
            m2nt = persist.tile([128, NKT], f32, tag="m2nt")
            nc.sync.dma_start(m2nt[:], m2n_d[:])
            ones1 = persist.tile([1, 1], bf16, tag="ones1")
            nc.vector.memset(ones1[:], 1.0)
            # dummy exp so the ~2.7us ACT table load overlaps the prep phase
            warm = persist.tile([1, 8], f32, tag="actwarm")
            nc.vector.memset(warm[:], 0.0)
            nc.scalar.activation(warm[:], warm[:], EXP)
            wq_r = wp_r[:, 0:192]
            wk_r = wp_r[:, 192:384]
            wv_r = wp_r[:, 384:576]
            wo_r = wp_r[0:64, 576:896]

            def wslice(wr, i):
                c0, cw = CCH[i]
                return wr[0:cw, i * 64:(i + 1) * 64]

            # ---- ctx^T / x^T (direct DMA, host-transposed, bf16) ---------
            ct = [persist.tile([128, M], bf16, tag="ct0", name="ct0"),
                  persist.tile([128, M], bf16, tag="ct1", name="ct1"),
                  persist.tile([64, M], bf16, tag="ct2", name="ct2")]

            # k row-packed for 2-way row tiling: partitions 0-63 hold d-dims
            # of even k-tiles, 64-127 of odd k-tiles; tile j's stationary is
            # kt2[64*(j%2):+64, (j//2)*128:+128].
            kt2 = persist.tile([128, M // 2], bf16, tag="kt2")
            vt = persist.tile([64, M], f32r, tag="vt")
            vaug = persist.tile([128, NKT, DA], bf16, tag="vaug")
            ones_col = persist.tile([128, NKT, 1], bf16, tag="ones_col")
            nc.vector.memset(ones_col[:], 1.0)
            nc.vector.tensor_copy(vaug[:, :, 64:65], ones_col[:])
            # q duplicated into both partition halves (moving operand for
            # row-tiled even/odd QK matmuls)
            qt2 = persist.tile([128, N], bf16, tag="qt2")
            assert QCHUNK == 512
            with (
                tc.tile_pool(name="sps", bufs=2, space="PSUM") as sps,
                tc.tile_pool(name="ops", bufs=1, space="PSUM") as ops,
                tc.tile_pool(name="mps", bufs=2, space="PSUM") as mps,
            ):
                kv_chunks = _chunks(M, 512)
                kv_next = [0]

                def emit_kv():
                    o, w = kv_chunks[kv_next[0]]
                    kv_next[0] += 1
                    for i, (c0, cw) in enumerate(CCH):
                        nc.gpsimd.dma_start(ct[i][0:cw, o:o + w],
                                            ctxt_d[c0:c0 + cw, o:o + w])
                    kps = mps.tile([64, 512], f32, tag="sm", name="kps")
                    vps = mps.tile([64, 512], f32, tag="sm", name="vps")
                    for i in range(3):
                        nc.tensor.matmul(kps[:, 0:w], wslice(wk_r, i),
                                         ct[i][0:CCH[i][1], o:o + w],
                                         start=(i == 0), stop=(i == 2))
                        nc.tensor.matmul(vps[:, 0:w], wslice(wv_r, i),
                                         ct[i][0:CCH[i][1], o:o + w],
                                         start=(i == 0), stop=(i == 2))
                    # scatter k 128-blocks into the row-packed layout.
                    # DVE copies are lane-wise (cannot shift partition base),
                    # so cast once to SBUF then move blocks with SBUF->SBUF
                    # DMA, which can place odd tiles at partitions 64-127.
                    ktmp = stage.tile([64, 512], bf16, tag="ktmp", bufs=2)
                    nc.vector.tensor_copy(ktmp[:, 0:w], kps[:, 0:w])
                    for b in range(w // 128):
                        j = o // 128 + b          # absolute k-tile index
                        half = 64 * (j % 2)
                        nc.sync.dma_start(
                            kt2[half:half + 64,
                                (j // 2) * 128:(j // 2) * 128 + 128],
                            ktmp[:, b * 128:(b + 1) * 128])
                    nc.vector.tensor_copy(vt[:, o:o + w], vps[:, 0:w])
                    for j in range(o // 128, (o + w) // 128):
                        vp = mps.tile([128, 64], f32r, tag="sm", name="vp")
                        nc.tensor.transpose(vp[:], vt[:, j * 128:(j + 1) * 128],
                                            eye[:])
                        nc.vector.tensor_copy(vaug[:, j, 0:64], vp[:])

                qprep_chunks = _chunks(N, 512)
                qprep_next = [0]

                def emit_qprep():
                    qo, qw = qprep_chunks[qprep_next[0]]
                    qprep_next[0] += 1
                    xt = [qpool.tile([128, 512], bf16, tag="xt0", name="xt0"),
                          qpool.tile([128, 512], bf16, tag="xt1", name="xt1"),
                          qpool.tile([64, 512], bf16, tag="xt2", name="xt2")]
                    for i, (c0, cw) in enumerate(CCH):
                        nc.gpsimd.dma_start(xt[i][0:cw, 0:qw],
                                            xt_d[c0:c0 + cw, qo:qo + qw])
                    qp = mps.tile([64, 512], f32, tag="sm", name="qp")
                    for i in range(3):
                        nc.tensor.matmul(qp[0:64, 0:qw], wslice(wq_r, i),
                                         xt[i][0:CCH[i][1], 0:qw],
                                         start=(i == 0), stop=(i == 2))
                    nc.vector.tensor_copy(qt2[0:64, qo:qo + qw], qp[0:64, 0:qw])
                    # duplicate into partitions 64-127 (moving operand for the
                    # odd row-tile) -- cross-partition, so SBUF->SBUF DMA
                    nc.sync.dma_start(qt2[64:128, qo:qo + qw],
                                      qt2[0:64, qo:qo + qw])

                pending_epi = [None]
                chunk_list = _chunks(n0, QCHUNK) + _chunks(N - n0, QCHUNK, n0)
                for (qo, qw) in chunk_list:
                    # keep q-prep one main-chunk ahead of consumption
                    target = min(N, qo + qw + QCHUNK)
                    while (qprep_next[0] < len(qprep_chunks)
                           and qprep_chunks[qprep_next[0]][0] < target):
                        emit_qprep()
                    masked = qo >= n0
                    nkt_c = NKT_SHORT if masked else NKT

                    # -- attention over k-tile pairs -----------------------
                    # A@V is split into k-partition halves on row-tiles T0/T8
                    # (same 64x128 PE mode as the QK matmuls -- no mode-switch
                    # drains, and LDWEIGHTS on one tile hides under the other
                    # tile's matmul).  oTa accumulates k 0-63 of every k-tile,
                    # oTb k 64-127; summed once per chunk on the DVE.
                    # The AV of pair i is emitted together with the QK of
                    # pair i+2 (software pipeline): the first AV -- which
                    # must wait for the previous chunk's oT banks to drain
                    # through the DVE -- then sits deep enough in the tensor
                    # queue that the PE never idles at the chunk boundary.
                    oTa = ops.tile([DA, QCHUNK], f32, tag="oTa", name="oTa")
                    oTb = ops.tile([DA, QCHUNK], f32, tag="oTb", name="oTb")
                    pair_list = []
                    jj = 0
                    while jj < nkt_c:
                        pair_list.append((jj, min(2, nkt_c - jj)))
                        jj += pair_list[-1][1]
                    at_tiles = {}

                    def emit_qk_exp(i):
                        jj, pair = pair_list[i]
                        while (kv_next[0] < len(kv_chunks)
                               and kv_next[0] * 4 < min(nkt_c, jj + 8)):
                            emit_kv()
                        s_ps = sps.tile([128, 1024], f32, tag="s")
                        for p in range(pair):
                            j = jj + p
                            half = 64 * (j % 2)
                            nc.tensor.matmul(
                                s_ps[:, p * 512:p * 512 + qw],
                                kt2[half:half + 64,
                                    (j // 2) * 128:(j // 2) * 128 + 128],
                                qt2[half:half + 64, qo:qo + qw],
                                start=True, stop=True)
                        at = apool.tile([128, 1024], bf16, tag="attn")
                        needs_bias = (masked and bias_tile is not None
                                      and jj <= bias_tile < jj + pair)
                        if pair == 2 and qw == 512 and not needs_bias:
                            nc.scalar.activation(at[:, 0:1024], s_ps[:, 0:1024],
                                                 EXP)
                        else:
                            for p in range(pair):
                                j = jj + p
                                if masked and j == bias_tile:
                                    nc.scalar.activation(
                                        at[:, p * 512:p * 512 + qw],
                                        s_ps[:, p * 512:p * 512 + qw], EXP,
                                        bias=m2nt[:, j:j + 1])
                                else:
                                    nc.scalar.activation(
                                        at[:, p * 512:p * 512 + qw],
                                        s_ps[:, p * 512:p * 512 + qw], EXP)
                        at_tiles[i] = at

                    def emit_av(i):
                        jj, pair = pair_list[i]
                        at = at_tiles.pop(i)
                        for p in range(pair):
                            j = jj + p
                            st = (j == 0)
                            sp = (j == nkt_c - 1)
                            nc.tensor.matmul(oTa[:, 0:qw], vaug[0:64, j, :],
                                             at[0:64, p * 512:p * 512 + qw],
                                             start=st, stop=sp)
                            nc.tensor.matmul(oTb[:, 0:qw], vaug[64:128, j, :],
                                             at[64:128, p * 512:p * 512 + qw],
                                             start=st, stop=sp)

                    npair = len(pair_list)
                    for i in range(min(2, npair)):
                        emit_qk_exp(i)
                    for i in range(2, npair):
                        emit_av(i - 2)
                        if i == 2 and pending_epi[0] is not None:
                            pending_epi[0]()
                            pending_epi[0] = None
                        emit_qk_exp(i)
                    for i in range(max(0, npair - 2), npair):
                        emit_av(i)

                    # -- epilogue part 1: drain oTa+oTb so the next chunk
                    # can start (DVE may read only ONE operand from PSUM,
                    # so stage oTb through SBUF first)
                    obs = ocpool.tile([DA, QCHUNK], f32, tag="obs",
                                      name="obs")
                    nc.vector.tensor_copy(obs[:, 0:qw], oTb[:, 0:qw])
                    oc = ocpool.tile([DA, QCHUNK], bf16, tag="oc", name="oc")
                    nc.vector.tensor_add(oc[:, 0:qw], oTa[:, 0:qw],
                                         obs[:, 0:qw])
                    srow = stage.tile([1, QCHUNK], f32, tag="srow")
                    nc.vector.tensor_add(srow[0:1, 0:qw], oTa[64:65, 0:qw],
                                         obs[64:65, 0:qw])

                    # reciprocal on the [1, qw] rowsum row, cast to bf16;
                    # the per-128-block transpose matmuls are then single
                    # cheap bf16 MMs instead of fp32 LOW_HIGH pairs.
                    rrow = stage.tile([1, QCHUNK], f32, tag="rrow")
                    nc.vector.reciprocal(rrow[0:1, 0:qw], srow[0:1, 0:qw])
                    rrow_bf = stage.tile([1, QCHUNK], bf16, tag="rrowbf")
                    nc.vector.tensor_copy(rrow_bf[0:1, 0:qw], rrow[0:1, 0:qw])

                    def epilogue(qo=qo, qw=qw, oc=oc, rrow_bf=rrow_bf):
                        nqt = -(-qw // 128)
                        rps = mps.tile([128, 8], f32, tag="sm", name="rps")
                        for t in range(nqt):
                            rem = min(128, qw - t * 128)
                            nc.tensor.matmul(rps[0:rem, t:t + 1],
                                             rrow_bf[0:1, t * 128:t * 128 + rem],
                                             ones1[0:1, 0:1],
                                             start=True, stop=True)
                        recip = stage.tile([128, 8], f32, tag="recip")
                        nc.vector.tensor_copy(recip[:, 0:nqt], rps[:, 0:nqt])
                        for t in range(nqt):
                            rem = min(128, qw - t * 128)
                            pps2 = mps.tile([128, 320], f32, tag="sm",
                                            name="pps2")
                            nc.tensor.matmul(pps2[0:rem, :],
                                             oc[0:64, t * 128:t * 128 + rem],
                                             wo_r[:], start=True, stop=True)
                            ot_sb = outsb.tile([128, 320], f32, tag="osb")
                            nc.vector.tensor_scalar_mul(ot_sb[0:rem, :],
                                                        pps2[0:rem, :],
                                                        recip[0:rem, t:t + 1])
                            nc.sync.dma_start(
                                out_d[qo + t * 128:qo + t * 128 + rem, :],
                                ot_sb[0:rem, :])

                    if pending_epi[0] is not None:
                        pending_epi[0]()
                    pending_epi[0] = epilogue
                if pending_epi[0] is not None:
                    pending_epi[0]()
                    pending_epi[0] = None

    nc.compile()
    return nc


def _get_compiled(n0=N, m0=M):
    key = (n0, m0)
    if key not in _compiled:
        _compiled[key] = _build_program(n0=n0, m0=m0)
    return _compiled[key]


def kernel(x, context, mask1, mask2, Wq, Wk, Wv, Wo, bo):
    from concourse import bass_utils
    import ml_dtypes

    global _last_in_maps, _last_key

    bf16 = ml_dtypes.bfloat16
    x = np.asarray(x, dtype=np.float32)
    context = np.asarray(context, dtype=np.float32)
    mask1 = np.asarray(mask1, dtype=np.float32)
    mask2 = np.asarray(mask2, dtype=np.float32)
    Wq = np.asarray(Wq, dtype=np.float32)
    Wk = np.asarray(Wk, dtype=np.float32)
    Wv = np.asarray(Wv, dtype=np.float32)
    Wo = np.asarray(Wo, dtype=np.float32)
    bo = np.asarray(bo, dtype=np.float32)

    b = x.shape[0]
    assert b == 1 and x.shape[1] == N and context.shape[1] == M

    # nearest-resize masks exactly as the reference does
    dxq = int((N // 12) ** 0.5)
    mH, mW = 4 * dxq, 3 * dxq
    dxk = int((M // 12) ** 0.5)
    mh, mw = 4 * dxk, 3 * dxk
    Hm, Wm = mask1.shape[-2], mask1.shape[-1]
    m1 = mask1[0, 0][(np.arange(mH) * Hm) // mH][:, (np.arange(mW) * Wm) // mW] >= 0.5
    m2 = mask2[0, 0][(np.arange(mh) * Hm) // mh][:, (np.arange(mw) * Wm) // mw] >= 0.5

    m1f = m1.reshape(-1)
    m2f = m2.reshape(-1)

    # group unmasked rows/cols first -> every q chunk is pure, and masked
    # chunks use a short k loop
    qperm = np.argsort(m1f, kind="stable")       # False (unmasked) first
    kperm = np.argsort(m2f, kind="stable")
    n0 = int((~m1f).sum())
    m0 = int((~m2f).sum())
    if n0 == N or m0 == 0:
        # no masked q rows (or degenerate mask): single dense region
        qperm = np.arange(N)
        kperm = np.arange(M)
        n0, m0 = N, M

    m2neg = np.where(m2f[kperm], np.float32(NEG), np.float32(0.0))
    xT = np.ascontiguousarray(x[0].T[:, qperm]).astype(bf16)
    ctxT = np.ascontiguousarray(context[0].T[:, kperm]).astype(bf16)

    def pack3(w):
        # [320, 64] -> [128, 192] (c-tiles of 128/128/64 side by side)
        p = np.zeros((128, 192), np.float32)
        p[:, 0:64] = w[0:128]
        p[:, 64:128] = w[128:256]
        p[0:64, 128:192] = w[256:320]
        return p

    def wpack(h):
        p = np.zeros((128, 896), np.float32)
        p[:, 0:192] = pack3(Wq[:, h * D:(h + 1) * D] * np.float32(SCALE))
        p[:, 192:384] = pack3(Wk[:, h * D:(h + 1) * D])
        p[:, 384:576] = pack3(Wv[:, h * D:(h + 1) * D])
        p[0:64, 576:896] = Wo[h * D:(h + 1) * D, :]
        return p.astype(bf16)

    m2n_tiles = m2neg.reshape(NKT, 128).T.copy()  # [128, NKT] fp32
    eye = np.eye(64, dtype=np.float32)

    in_maps = []
    for h in range(HEADS):
        in_maps.append({
            "xt": xT,
            "ctxt": ctxT,
            "wpack": wpack(h),
            "eye": eye,
            "m2neg": m2n_tiles,
        })
    _last_in_maps = in_maps
    _last_key = (n0, m0)

    nc = _get_compiled(n0, m0)
    res = bass_utils.run_bass_kernel_spmd(nc, in_maps, list(range(HEADS)))
    out = np.zeros((N, C), dtype=np.float32)
    for h in range(HEADS):
        out += res.results[h]["out"]
    out += bo
    inv = np.empty(N, dtype=np.int64)
    inv[qperm] = np.arange(N)
    out = out[inv]
    return out.reshape(1, N, C)


# revision 12
# speedup vs baseline: 1.1946x; 1.1946x over previous
"""Trainium2 Bass kernel for nn_CrossAttention_43258910605402.

Masked cross-attention, head-parallel over 8 NeuronCores (one head per core).

Math (per head h):
  q = x @ Wq[:, 64h:64h+64] * d^-0.5          [n=6912, 64]
  k = ctx @ Wk[:, 64h:64h+64]                 [m=3072, 64]
  v = ctx @ Wv[:, 64h:64h+64]                 [m=3072, 64]
  S = q @ k^T  (masked entries -> -inf)       [n, m]
  A = exp(S)   (no row-max: |S| <= ~10 here; masked -> exp = 0)
  out_h = (A @ v) / rowsum(A)                 [n, 64]
  partial = out_h @ Wo[64h:64h+64, :]         [n, 320]
Host: out = sum_h partial_h + bo.

The mask is rank-1 (m1_i & m2_j).  Host permutes q rows and k cols so
unmasked entries come first (n0 / m0 split).  Chunks of q are then PURE:
unmasked chunks attend all of k with no mask at all; masked chunks attend
only the first ceil(m0/128) k-tiles, and only the last (partial) k-tile
needs masking, applied as a per-partition bias (-1e30) on the exp
activation -- so the contraction stays exactly 64.

With K=64 the QK^T matmuls run 2-way row-tiled: even k-tiles occupy PE
rows 0-63 (stationary at SBUF partitions 0-63), odd k-tiles rows 64-127,
executing concurrently into the two halves of one [128, 1024] PSUM pair.
One exp ACTIVATE covers the pair.  A@V uses a [128, 65] stationary
(v plus a ones column that yields rowsum(A) as output row 64);
normalization is deferred into a per-partition scalar multiply after the
output projection.  All matmul operands are bf16 (fp32 PSUM accumulate),
which also halves input DMA.
"""

import numpy as np

HEADS = 8
D = 64
DA = 65          # d + 1 ones row for fused rowsum
N = 6912         # query positions
M = 3072         # key positions
C = 320          # model dim
SCALE = D ** -0.5
NEG = -1e30
NKT = M // 128

_compiled = {}
_last_in_maps = None
_last_key = None


def _chunks(total, size, base=0):
    out = []
    o = 0
    while o < total:
        w = min(size, total - o)
        out.append((base + o, w))
        o += w
    return out


def _build_program(n0=N, m0=M, QCHUNK=512):
    # n0: q rows 0..n0-1 are unmasked (attend all k); rows n0.. attend only
    # k < m0.  NKT_SHORT = ceil(m0/128) k-tiles, with a -1e30 per-partition
    # exp bias killing the masked tail of the last short tile.
    import concourse.bacc as bacc
    import concourse.tile as tile
    import concourse.mybir as mybir

    NKT_SHORT = max(1, -(-m0 // 128))
    bias_tile = NKT_SHORT - 1 if (m0 % 128 != 0 and n0 < N) else None
    f32 = mybir.dt.float32
    f32r = mybir.dt.float32r
    bf16 = mybir.dt.bfloat16
    EXP = mybir.ActivationFunctionType.Exp

    nc = bacc.Bacc("TRN2", target_bir_lowering=False, debug=False)

    xt_d = nc.dram_tensor("xt", [C, N], bf16, kind="ExternalInput").ap()
    ctxt_d = nc.dram_tensor("ctxt", [C, M], bf16, kind="ExternalInput").ap()
    # packed weights: [128, 896] bf16 = wq(192) wk(192) wv(192) | wo 64x320
    wp_d = nc.dram_tensor("wpack", [128, 896], bf16, kind="ExternalInput").ap()
    eye_d = nc.dram_tensor("eye", [64, 64], f32r, kind="ExternalInput").ap()
    m2n_d = nc.dram_tensor("m2neg", [128, NKT], f32, kind="ExternalInput").ap()
    out_d = nc.dram_tensor("out", [N, C], f32, kind="ExternalOutput").ap()

    CCH = [(0, 128), (128, 128), (256, 64)]   # contraction tiles over C=320

    with tile.TileContext(nc) as tc:
        with (
            tc.tile_pool(name="persist", bufs=1) as persist,
            tc.tile_pool(name="stage", bufs=3) as stage,
            tc.tile_pool(name="qpool", bufs=2) as qpool,
            tc.tile_pool(name="attn", bufs=3) as apool,
            tc.tile_pool(name="oc", bufs=2) as ocpool,
            tc.tile_pool(name="outsb", bufs=3) as outsb,
        ):
            # ---- constants / weights -------------------------------------
            wp_r = persist.tile([128, 896], bf16, tag="wpack")
            nc.sync.dma_start(wp_r[:], wp_d[:])
            eye = persist.tile([64, 64], f32r, tag="eye")
            nc.sync.dma_start(eye[:], eye_d[:])
            m2nt = persist.tile([128, NKT], f32, tag="m2nt")
            nc.sync.dma_start(m2nt[:], m2n_d[:])
            ones1 = persist.tile([1, 1], bf16, tag="ones1")
            nc.vector.memset(ones1[:], 1.0)
            # dummy exp so the ~2.7us ACT table load overlaps the prep phase
            warm = persist.tile([1, 8], f32, tag="actwarm")
            nc.vector.memset(warm[:], 0.0)
            nc.scalar.activation(warm[:], warm[:], EXP)
            wq_r = wp_r[:, 0:192]
            wk_r = wp_r[:, 192:384]
            wv_r = wp_r[:, 384:576]
            wo_r = wp_r[0:64, 576:896]

            def wslice(wr, i):
                c0, cw = CCH[i]
                return wr[0:cw, i * 64:(i + 1) * 64]

            # ---- ctx^T / x^T (direct DMA, host-transposed, bf16) ---------
            ct = [persist.tile([128, M], bf16, tag="ct0", name="ct0"),
                  persist.tile([128, M], bf16, tag="ct1", name="ct1"),
                  persist.tile([64, M], bf16, tag="ct2", name="ct2")]

            # k row-packed for 2-way row tiling: partitions 0-63 hold d-dims
            # of even k-tiles, 64-127 of odd k-tiles; tile j's stationary is
            # kt2[64*(j%2):+64, (j//2)*128:+128].
            kt2 = persist.tile([128, M // 2], bf16, tag="kt2")
            vt = persist.tile([64, M], f32r, tag="vt")
            vaug = persist.tile([128, NKT, DA], bf16, tag="vaug")
            ones_col = persist.tile([128, NKT, 1], bf16, tag="ones_col")
            nc.vector.memset(ones_col[:], 1.0)
            nc.vector.tensor_copy(vaug[:, :, 64:65], ones_col[:])
            # q duplicated into both partition halves (moving operand for
            # row-tiled even/odd QK matmuls)
            qt2 = persist.tile([128, N], bf16, tag="qt2")
            assert QCHUNK == 512
            with (
                tc.tile_pool(name="sps", bufs=2, space="PSUM") as sps,
                tc.tile_pool(name="ops", bufs=1, space="PSUM") as ops,
                tc.tile_pool(name="mps", bufs=2, space="PSUM") as mps,
            ):
                kv_chunks = _chunks(M, 512)
                kv_next = [0]

                def emit_kv():
                    o, w = kv_chunks[kv_next[0]]
                    kv_next[0] += 1
                    for i, (c0, cw) in enumerate(CCH):
                        nc.gpsimd.dma_start(ct[i][0:cw, o:o + w],
                                            ctxt_d[c0:c0 + cw, o:o + w])
                    kps = mps.tile([64, 512], f32, tag="sm", name="kps")
                    vps = mps.tile([64, 512], f32, tag="sm", name="vps")
                    for i in range(3):
                        nc.tensor.matmul(kps[:, 0:w], wslice(wk_r, i),
                                         ct[i][0:CCH[i][1], o:o + w],
                                         start=(i == 0), stop=(i == 2))
                        nc.tensor.matmul(vps[:, 0:w], wslice(wv_r, i),
                                         ct[i][0:CCH[i][1], o:o + w],
                                         start=(i == 0), stop=(i == 2))
                    # scatter k 128-blocks into the row-packed layout.
                    # DVE copies are lane-wise (cannot shift partition base),
                    # so cast once to SBUF then move blocks with SBUF->SBUF
                    # DMA, which can place odd tiles at partitions 64-127.
                    ktmp = stage.tile([64, 512], bf16, tag="ktmp", bufs=2)
                    nc.vector.tensor_copy(ktmp[:, 0:w], kps[:, 0:w])
                    for b in range(w // 128):
                        j = o // 128 + b          # absolute k-tile index
                        half = 64 * (j % 2)
                        nc.sync.dma_start(
                            kt2[half:half + 64,
                                (j // 2) * 128:(j // 2) * 128 + 128],
                            ktmp[:, b * 128:(b + 1) * 128])
                    nc.vector.tensor_copy(vt[:, o:o + w], vps[:, 0:w])
                    for j in range(o // 128, (o + w) // 128):
                        vp = mps.tile([128, 64], f32r, tag="sm", name="vp")
                        nc.tensor.transpose(vp[:], vt[:, j * 128:(j + 1) * 128],
                                            eye[:])
                        nc.vector.tensor_copy(vaug[:, j, 0:64], vp[:])

                qprep_chunks = _chunks(N, 512)
                qprep_next = [0]

                def emit_qprep():
                    qo, qw = qprep_chunks[qprep_next[0]]
                    qprep_next[0] += 1
                    xt = [qpool.tile([128, 512], bf16, tag="xt0", name="xt0"),
                          qpool.tile([128, 512], bf16, tag="xt1", name="xt1"),
                          qpool.tile([64, 512], bf16, tag="xt2", name="xt2")]
                    for i, (c0, cw) in enumerate(CCH):
                        nc.gpsimd.dma_start(xt[i][0:cw, 0:qw],
                                            xt_d[c0:c0 + cw, qo:qo + qw])
                    qp = mps.tile([64, 512], f32, tag="sm", name="qp")
                    for i in range(3):
                        nc.tensor.matmul(qp[0:64, 0:qw], wslice(wq_r, i),
                                         xt[i][0:CCH[i][1], 0:qw],
                                         start=(i == 0), stop=(i == 2))
                    nc.vector.tensor_copy(qt2[0:64, qo:qo + qw], qp[0:64, 0:qw])
                    # duplicate into partitions 64-127 (moving operand for the
                    # odd row-tile) -- cross-partition, so SBUF->SBUF DMA
                    nc.sync.dma_start(qt2[64:128, qo:qo + qw],
                                      qt2[0:64, qo:qo + qw])

                pending_epi = [None]
                chunk_list = _chunks(n0, QCHUNK) + _chunks(N - n0, QCHUNK, n0)
                for (qo, qw) in chunk_list:
                    # keep q-prep one main-chunk ahead of consumption
                    target = min(N, qo + qw + QCHUNK)
                    while (qprep_next[0] < len(qprep_chunks)
                           and qprep_chunks[qprep_next[0]][0] < target):
                        emit_qprep()
                    masked = qo >= n0
                    nkt_c = NKT_SHORT if masked else NKT

                    # -- attention over k-tile pairs -----------------------
                    # A@V is split into k-partition halves on row-tiles T0/T8
                    # (same 64x128 PE mode as the QK matmuls -- no mode-switch
                    # drains, and LDWEIGHTS on one tile hides under the other
                    # tile's matmul).  oTa accumulates k 0-63 of every k-tile,
                    # oTb k 64-127; summed once per chunk on the DVE.
                    # The AV of pair i is emitted together with the QK of
                    # pair i+2 (software pipeline): the first AV -- which
                    # must wait for the previous chunk's oT banks to drain
                    # through the DVE -- then sits deep enough in the tensor
                    # queue that the PE never idles at the chunk boundary.
                    oTa = ops.tile([DA, QCHUNK], f32, tag="oTa", name="oTa")
                    oTb = ops.tile([DA, QCHUNK], f32, tag="oTb", name="oTb")
                    pair_list = []
                    jj = 0
                    while jj < nkt_c:
                        pair_list.append((jj, min(2, nkt_c - jj)))
                        jj += pair_list[-1][1]
                    at_tiles = {}

                    def emit_qk_exp(i):
                        jj, pair = pair_list[i]
                        while (kv_next[0] < len(kv_chunks)
                               and kv_next[0] * 4 < min(nkt_c, jj + 8)):
                            emit_kv()
                        s_ps = sps.tile([128, 1024], f32, tag="s")
                        for p in range(pair):
                            j = jj + p
                            half = 64 * (j % 2)
                            nc.tensor.matmul(
                                s_ps[:, p * 512:p * 512 + qw],
                                kt2[half:half + 64,
                                    (j // 2) * 128:(j // 2) * 128 + 128],
                                qt2[half:half + 64, qo:qo + qw],
                                start=True, stop=True)
                        at = apool.tile([128, 1024], bf16, tag="attn")
                        needs_bias = (masked and bias_tile is not None
                                      and jj <= bias_tile < jj + pair)
                        if pair == 2 and qw == 512 and not needs_bias:
                            nc.scalar.activation(at[:, 0:1024], s_ps[:, 0:1024],
                                                 EXP)
                        else:
                            for p in range(pair):
                                j = jj + p
                                if masked and j == bias_tile:
                                    nc.scalar.activation(
                                        at[:, p * 512:p * 512 + qw],
                                        s_ps[:, p * 512:p * 512 + qw], EXP,
                                        bias=m2nt[:, j:j + 1])
                                else:
                                    nc.scalar.activation(
                                        at[:, p * 512:p * 512 + qw],
                                        s_ps[:, p * 512:p * 512 + qw], EXP)
                        at_tiles[i] = at

                    def emit_av(i):
                        jj, pair = pair_list[i]
                        at = at_tiles.pop(i)
                        for p in range(pair):
                            j = jj + p
                            st = (j == 0)
                            sp = (j == nkt_c - 1)
                            nc.tensor.matmul(oTa[:, 0:qw], vaug[0:64, j, :],
                                             at[0:64, p * 512:p * 512 + qw],
                                             start=st, stop=sp)
                            nc.tensor.matmul(oTb[:, 0:qw], vaug[64:128, j, :],
                                             at[64:128, p * 512:p * 512 + qw],
                                             start=st, stop=sp)

                    npair = len(pair_list)
                    for i in range(min(2, npair)):
                        emit_qk_exp(i)
                    for i in range(2, npair):
                        emit_av(i - 2)
                        if i == 2 and pending_epi[0] is not None:
                            pending_epi[0]()
                            pending_epi[0] = None
                        emit_qk_exp(i)
                    for i in range(max(0, npair - 2), npair):
                        emit_av(i)

                    # -- epilogue part 1: drain oTa+oTb so the next chunk
                    # can start (DVE may read only ONE operand from PSUM,
                    # so stage oTb through SBUF first)
                    obs = ocpool.tile([DA, QCHUNK], f32, tag="obs",
                                      name="obs")
                    nc.vector.tensor_copy(obs[:, 0:qw], oTb[:, 0:qw])
                    oc = ocpool.tile([DA, QCHUNK], bf16, tag="oc", name="oc")
                    nc.vector.tensor_add(oc[:, 0:qw], oTa[:, 0:qw],
                                         obs[:, 0:qw])
                    srow = stage.tile([1, QCHUNK], f32, tag="srow")
                    nc.vector.tensor_add(srow[0:1, 0:qw], oTa[64:65, 0:qw],
                                         obs[64:65, 0:qw])

                    # cast the rowsum row to bf16 so the per-128-block
                    # transpose matmuls are single cheap bf16 MMs instead of
                    # fp32 LOW_HIGH pairs; reciprocal happens after the
                    # transpose on a tiny [128, nqt] tile (reciprocal cost
                    # scales with free size).
                    srow_bf = stage.tile([1, QCHUNK], bf16, tag="srowbf")
                    nc.vector.tensor_copy(srow_bf[0:1, 0:qw], srow[0:1, 0:qw])

                    def epilogue(qo=qo, qw=qw, oc=oc, srow_bf=srow_bf):
                        nqt = -(-qw // 128)
                        rps = mps.tile([128, 8], f32, tag="sm", name="rps")
                        for t in range(nqt):
                            rem = min(128, qw - t * 128)
                            nc.tensor.matmul(rps[0:rem, t:t + 1],
                                             srow_bf[0:1, t * 128:t * 128 + rem],
                                             ones1[0:1, 0:1],
                                             start=True, stop=True)
                        recip = stage.tile([128, 8], f32, tag="recip")
                        nc.vector.reciprocal(recip[:, 0:nqt], rps[:, 0:nqt])
                        for t in range(nqt):
                            rem = min(128, qw - t * 128)
                            pps2 = mps.tile([128, 320], f32, tag="sm",
                                            name="pps2")
                            nc.tensor.matmul(pps2[0:rem, :],
                                             oc[0:64, t * 128:t * 128 + rem],
                                             wo_r[:], start=True, stop=True)
                            ot_sb = outsb.tile([128, 320], f32, tag="osb")
                            nc.vector.tensor_scalar_mul(ot_sb[0:rem, :],
                                                        pps2[0:rem, :],
                                                        recip[0:rem, t:t + 1])
                            nc.sync.dma_start(
                                out_d[qo + t * 128:qo + t * 128 + rem, :],
                                ot_sb[0:rem, :])

                    if pending_epi[0] is not None:
                        pending_epi[0]()
                    pending_epi[0] = epilogue
                if pending_epi[0] is not None:
                    pending_epi[0]()
                    pending_epi[0] = None

    nc.compile()
    return nc


def _get_compiled(n0=N, m0=M):
    key = (n0, m0)
    if key not in _compiled:
        _compiled[key] = _build_program(n0=n0, m0=m0)
    return _compiled[key]


def kernel(x, context, mask1, mask2, Wq, Wk, Wv, Wo, bo):
    from concourse import bass_utils
    import ml_dtypes

    global _last_in_maps, _last_key

    bf16 = ml_dtypes.bfloat16
    x = np.asarray(x, dtype=np.float32)
    context = np.asarray(context, dtype=np.float32)
    mask1 = np.asarray(mask1, dtype=np.float32)
    mask2 = np.asarray(mask2, dtype=np.float32)
    Wq = np.asarray(Wq, dtype=np.float32)
    Wk = np.asarray(Wk, dtype=np.float32)
    Wv = np.asarray(Wv, dtype=np.float32)
    Wo = np.asarray(Wo, dtype=np.float32)
    bo = np.asarray(bo, dtype=np.float32)

    b = x.shape[0]
    assert b == 1 and x.shape[1] == N and context.shape[1] == M

    # nearest-resize masks exactly as the reference does
    dxq = int((N // 12) ** 0.5)
    mH, mW = 4 * dxq, 3 * dxq
    dxk = int((M // 12) ** 0.5)
    mh, mw = 4 * dxk, 3 * dxk
    Hm, Wm = mask1.shape[-2], mask1.shape[-1]
    m1 = mask1[0, 0][(np.arange(mH) * Hm) // mH][:, (np.arange(mW) * Wm) // mW] >= 0.5
    m2 = mask2[0, 0][(np.arange(mh) * Hm) // mh][:, (np.arange(mw) * Wm) // mw] >= 0.5

    m1f = m1.reshape(-1)
    m2f = m2.reshape(-1)

    # group unmasked rows/cols first -> every q chunk is pure, and masked
    # chunks use a short k loop
    qperm = np.argsort(m1f, kind="stable")       # False (unmasked) first
    kperm = np.argsort(m2f, kind="stable")
    n0 = int((~m1f).sum())
    m0 = int((~m2f).sum())
    if n0 == N or m0 == 0:
        # no masked q rows (or degenerate mask): single dense region
        qperm = np.arange(N)
        kperm = np.arange(M)
        n0, m0 = N, M

    m2neg = np.where(m2f[kperm], np.float32(NEG), np.float32(0.0))
    xT = np.ascontiguousarray(x[0].T[:, qperm]).astype(bf16)
    ctxT = np.ascontiguousarray(context[0].T[:, kperm]).astype(bf16)

    def pack3(w):
        # [320, 64] -> [128, 192] (c-tiles of 128/128/64 side by side)
        p = np.zeros((128, 192), np.float32)
        p[:, 0:64] = w[0:128]
        p[:, 64:128] = w[128:256]
        p[0:64, 128:192] = w[256:320]
        return p

    def wpack(h):
        p = np.zeros((128, 896), np.float32)
        p[:, 0:192] = pack3(Wq[:, h * D:(h + 1) * D] * np.float32(SCALE))
        p[:, 192:384] = pack3(Wk[:, h * D:(h + 1) * D])
        p[:, 384:576] = pack3(Wv[:, h * D:(h + 1) * D])
        p[0:64, 576:896] = Wo[h * D:(h + 1) * D, :]
        return p.astype(bf16)

    m2n_tiles = m2neg.reshape(NKT, 128).T.copy()  # [128, NKT] fp32
    eye = np.eye(64, dtype=np.float32)

    in_maps = []
    for h in range(HEADS):
        in_maps.append({
            "xt": xT,
            "ctxt": ctxT,
            "wpack": wpack(h),
            "eye": eye,
            "m2neg": m2n_tiles,
        })
    _last_in_maps = in_maps
    _last_key = (n0, m0)

    nc = _get_compiled(n0, m0)
    res = bass_utils.run_bass_kernel_spmd(nc, in_maps, list(range(HEADS)))
    out = np.zeros((N, C), dtype=np.float32)
    for h in range(HEADS):
        out += res.results[h]["out"]
    out += bo
    inv = np.empty(N, dtype=np.int64)
    inv[qperm] = np.arange(N)
    out = out[inv]
    return out.reshape(1, N, C)


# revision 16
# speedup vs baseline: 1.2228x; 1.0237x over previous
"""Trainium2 Bass kernel for nn_CrossAttention_43258910605402.

Masked cross-attention, head-parallel over 8 NeuronCores (one head per core).

Math (per head h):
  q = x @ Wq[:, 64h:64h+64] * d^-0.5          [n=6912, 64]
  k = ctx @ Wk[:, 64h:64h+64]                 [m=3072, 64]
  v = ctx @ Wv[:, 64h:64h+64]                 [m=3072, 64]
  S = q @ k^T  (masked entries -> -inf)       [n, m]
  A = exp(S)   (no row-max: |S| <= ~10 here; masked -> exp = 0)
  out_h = (A @ v) / rowsum(A)                 [n, 64]
  partial = out_h @ Wo[64h:64h+64, :]         [n, 320]
Host: out = sum_h partial_h + bo.

The mask is rank-1 (m1_i & m2_j).  Host permutes q rows and k cols so
unmasked entries come first (n0 / m0 split).  Chunks of q are then PURE:
unmasked chunks attend all of k with no mask at all; masked chunks attend
only the first ceil(m0/128) k-tiles, and only the last (partial) k-tile
needs masking, applied as a per-partition bias (-1e30) on the exp
activation -- so the contraction stays exactly 64.

With K=64 the QK^T matmuls run 2-way row-tiled: even k-tiles occupy PE
rows 0-63 (stationary at SBUF partitions 0-63), odd k-tiles rows 64-127,
executing concurrently into the two halves of one [128, 1024] PSUM pair.
One exp ACTIVATE covers the pair.  A@V uses a [128, 65] stationary
(v plus a ones column that yields rowsum(A) as output row 64);
normalization is deferred into a per-partition scalar multiply after the
output projection.  All matmul operands are bf16 (fp32 PSUM accumulate),
which also halves input DMA.
"""

import numpy as np

HEADS = 8
D = 64
DA = 65          # d + 1 ones row for fused rowsum
N = 6912         # query positions
M = 3072         # key positions
C = 320          # model dim
SCALE = D ** -0.5
NEG = -1e30
NKT = M // 128

_compiled = {}
_last_in_maps = None
_last_key = None


def _chunks(total, size, base=0):
    out = []
    o = 0
    while o < total:
        w = min(size, total - o)
        out.append((base + o, w))
        o += w
    return out


def _build_program(n0=N, m0=M, QCHUNK=512):
    # n0: q rows 0..n0-1 are unmasked (attend all k); rows n0.. attend only
    # k < m0.  NKT_SHORT = ceil(m0/128) k-tiles, with a -1e30 per-partition
    # exp bias killing the masked tail of the last short tile.
    import concourse.bacc as bacc
    import concourse.tile as tile
    import concourse.mybir as mybir

    NKT_SHORT = max(1, -(-m0 // 128))
    bias_tile = NKT_SHORT - 1 if (m0 % 128 != 0 and n0 < N) else None
    f32 = mybir.dt.float32
    f32r = mybir.dt.float32r
    bf16 = mybir.dt.bfloat16
    EXP = mybir.ActivationFunctionType.Exp

    nc = bacc.Bacc("TRN2", target_bir_lowering=False, debug=False)

    xt_d = nc.dram_tensor("xt", [C, N], bf16, kind="ExternalInput").ap()
    ctxt_d = nc.dram_tensor("ctxt", [C, M], bf16, kind="ExternalInput").ap()
    # packed weights: [128, 896] bf16 = wq(192) wk(192) wv(192) | wo 64x320
    wp_d = nc.dram_tensor("wpack", [128, 896], bf16, kind="ExternalInput").ap()
    eye_d = nc.dram_tensor("eye", [64, 64], f32r, kind="ExternalInput").ap()
    m2n_d = nc.dram_tensor("m2neg", [128, NKT], f32, kind="ExternalInput").ap()
    out_d = nc.dram_tensor("out", [N, C], f32, kind="ExternalOutput").ap()

    CCH = [(0, 128), (128, 128), (256, 64)]   # contraction tiles over C=320

    with tile.TileContext(nc) as tc:
        with (
            tc.tile_pool(name="persist", bufs=1) as persist,
            tc.tile_pool(name="stage", bufs=3) as stage,
            tc.tile_pool(name="qpool", bufs=2) as qpool,
            tc.tile_pool(name="attn", bufs=3) as apool,
            tc.tile_pool(name="oc", bufs=2) as ocpool,
            tc.tile_pool(name="outsb", bufs=3) as outsb,
        ):
            # ---- constants / weights -------------------------------------
            wp_r = persist.tile([128, 896], bf16, tag="wpack")
            nc.sync.dma_start(wp_r[:], wp_d[:])
            eye = persist.tile([64, 64], f32r, tag="eye")
            nc.sync.dma_start(eye[:], eye_d[:])
            m2nt = persist.tile([128, NKT], f32, tag="m2nt")
            nc.sync.dma_start(m2nt[:], m2n_d[:])
            ones1 = persist.tile([1, 1], bf16, tag="ones1")
            nc.vector.memset(ones1[:], 1.0)
            # dummy exp so the ~2.7us ACT table load overlaps the prep phase
            warm = persist.tile([1, 8], f32, tag="actwarm")
            nc.vector.memset(warm[:], 0.0)
            nc.scalar.activation(warm[:], warm[:], EXP)
            wq_r = wp_r[:, 0:192]
            wk_r = wp_r[:, 192:384]
            wv_r = wp_r[:, 384:576]
            wo_r = wp_r[0:64, 576:896]

            def wslice(wr, i):
                c0, cw = CCH[i]
                return wr[0:cw, i * 64:(i + 1) * 64]

            # ---- ctx^T / x^T (direct DMA, host-transposed, bf16) ---------
            ct = [persist.tile([128, M], bf16, tag="ct0", name="ct0"),
                  persist.tile([128, M], bf16, tag="ct1", name="ct1"),
                  persist.tile([64, M], bf16, tag="ct2", name="ct2")]

            # k row-packed for 2-way row tiling: partitions 0-63 hold d-dims
            # of even k-tiles, 64-127 of odd k-tiles; tile j's stationary is
            # kt2[64*(j%2):+64, (j//2)*128:+128].
            kt2 = persist.tile([128, M // 2], bf16, tag="kt2")
            vt = persist.tile([64, M], f32r, tag="vt")
            vaug = persist.tile([128, NKT, DA], bf16, tag="vaug")
            ones_col = persist.tile([128, NKT, 1], bf16, tag="ones_col")
            nc.vector.memset(ones_col[:], 1.0)
            nc.vector.tensor_copy(vaug[:, :, 64:65], ones_col[:])
            # q duplicated into both partition halves (moving operand for
            # row-tiled even/odd QK matmuls)
            qt2 = persist.tile([128, N], bf16, tag="qt2")
            assert QCHUNK == 512
            with (
                tc.tile_pool(name="sps", bufs=2, space="PSUM") as sps,
                tc.tile_pool(name="ops", bufs=2, space="PSUM") as ops,
                tc.tile_pool(name="mps", bufs=2, space="PSUM") as mps,
            ):
                kv_chunks = _chunks(M, 512)
                kv_next = [0]

                def emit_kv():
                    o, w = kv_chunks[kv_next[0]]
                    kv_next[0] += 1
                    for i, (c0, cw) in enumerate(CCH):
                        nc.gpsimd.dma_start(ct[i][0:cw, o:o + w],
                                            ctxt_d[c0:c0 + cw, o:o + w])
                    kps = mps.tile([64, 512], f32, tag="sm", name="kps")
                    vps = mps.tile([64, 512], f32, tag="sm", name="vps")
                    for i in range(3):
                        nc.tensor.matmul(kps[:, 0:w], wslice(wk_r, i),
                                         ct[i][0:CCH[i][1], o:o + w],
                                         start=(i == 0), stop=(i == 2))
                        nc.tensor.matmul(vps[:, 0:w], wslice(wv_r, i),
                                         ct[i][0:CCH[i][1], o:o + w],
                                         start=(i == 0), stop=(i == 2))
                    # scatter k 128-blocks into the row-packed layout.
                    # DVE copies are lane-wise (cannot shift partition base),
                    # so cast once to SBUF then move blocks with SBUF->SBUF
                    # DMA, which can place odd tiles at partitions 64-127.
                    ktmp = stage.tile([64, 512], bf16, tag="ktmp", bufs=2)
                    nc.vector.tensor_copy(ktmp[:, 0:w], kps[:, 0:w])
                    for b in range(w // 128):
                        j = o // 128 + b          # absolute k-tile index
                        half = 64 * (j % 2)
                        nc.sync.dma_start(
                            kt2[half:half + 64,
                                (j // 2) * 128:(j // 2) * 128 + 128],
                            ktmp[:, b * 128:(b + 1) * 128])
                    nc.vector.tensor_copy(vt[:, o:o + w], vps[:, 0:w])
                    for j in range(o // 128, (o + w) // 128):
                        vp = mps.tile([128, 64], f32r, tag="sm", name="vp")
                        nc.tensor.transpose(vp[:], vt[:, j * 128:(j + 1) * 128],
                                            eye[:])
                        nc.vector.tensor_copy(vaug[:, j, 0:64], vp[:])

                qprep_chunks = _chunks(N, 512)
                qprep_next = [0]

                def emit_qprep():
                    qo, qw = qprep_chunks[qprep_next[0]]
                    qprep_next[0] += 1
                    xt = [qpool.tile([128, 512], bf16, tag="xt0", name="xt0"),
                          qpool.tile([128, 512], bf16, tag="xt1", name="xt1"),
                          qpool.tile([64, 512], bf16, tag="xt2", name="xt2")]
                    for i, (c0, cw) in enumerate(CCH):
                        nc.gpsimd.dma_start(xt[i][0:cw, 0:qw],
                                            xt_d[c0:c0 + cw, qo:qo + qw])
                    qp = mps.tile([64, 512], f32, tag="sm", name="qp")
                    for i in range(3):
                        nc.tensor.matmul(qp[0:64, 0:qw], wslice(wq_r, i),
                                         xt[i][0:CCH[i][1], 0:qw],
                                         start=(i == 0), stop=(i == 2))
                    nc.vector.tensor_copy(qt2[0:64, qo:qo + qw], qp[0:64, 0:qw])
                    # duplicate into partitions 64-127 (moving operand for the
                    # odd row-tile) -- cross-partition, so SBUF->SBUF DMA
                    nc.sync.dma_start(qt2[64:128, qo:qo + qw],
                                      qt2[0:64, qo:qo + qw])

                pending_epi = [None]
                chunk_list = _chunks(n0, QCHUNK) + _chunks(N - n0, QCHUNK, n0)
                for (qo, qw) in chunk_list:
                    # keep q-prep one main-chunk ahead of consumption
                    target = min(N, qo + qw + QCHUNK)
                    while (qprep_next[0] < len(qprep_chunks)
                           and qprep_chunks[qprep_next[0]][0] < target):
                        emit_qprep()
                    masked = qo >= n0
                    nkt_c = NKT_SHORT if masked else NKT

                    # -- attention over k-tile pairs -----------------------
                    # The AV of pair i is emitted together with the QK of
                    # pair i+2 (software pipeline), so the PE never waits on
                    # the exp of the pair it just computed.  oT is
                    # double-buffered across chunks, so the first AV of a
                    # chunk starts while the previous chunk's oT drains.
                    oT = ops.tile([DA, QCHUNK], f32, tag="oT")
                    pair_list = []
                    jj = 0
                    while jj < nkt_c:
                        pair_list.append((jj, min(2, nkt_c - jj)))
                        jj += pair_list[-1][1]
                    at_tiles = {}

                    def emit_qk_exp(i):
                        jj, pair = pair_list[i]
                        while (kv_next[0] < len(kv_chunks)
                               and kv_next[0] * 4 < min(nkt_c, jj + 8)):
                            emit_kv()
                        s_ps = sps.tile([128, 1024], f32, tag="s")
                        for p in range(pair):
                            j = jj + p
                            half = 64 * (j % 2)
                            nc.tensor.matmul(
                                s_ps[:, p * 512:p * 512 + qw],
                                kt2[half:half + 64,
                                    (j // 2) * 128:(j // 2) * 128 + 128],
                                qt2[half:half + 64, qo:qo + qw],
                                start=True, stop=True)
                        at = apool.tile([128, 1024], bf16, tag="attn")
                        needs_bias = (masked and bias_tile is not None
                                      and jj <= bias_tile < jj + pair)
                        if pair == 2 and qw == 512 and not needs_bias:
                            nc.scalar.activation(at[:, 0:1024], s_ps[:, 0:1024],
                                                 EXP)
                        else:
                            for p in range(pair):
                                j = jj + p
                                if masked and j == bias_tile:
                                    nc.scalar.activation(
                                        at[:, p * 512:p * 512 + qw],
                                        s_ps[:, p * 512:p * 512 + qw], EXP,
                                        bias=m2nt[:, j:j + 1])
                                else:
                                    nc.scalar.activation(
                                        at[:, p * 512:p * 512 + qw],
                                        s_ps[:, p * 512:p * 512 + qw], EXP)
                        at_tiles[i] = at

                    def emit_av(i):
                        jj, pair = pair_list[i]
                        at = at_tiles.pop(i)
                        for p in range(pair):
                            j = jj + p
                            nc.tensor.matmul(oT[:, 0:qw], vaug[:, j, :],
                                             at[:, p * 512:p * 512 + qw],
                                             start=(j == 0),
                                             stop=(j == nkt_c - 1))

                    npair = len(pair_list)
                    for i in range(min(2, npair)):
                        emit_qk_exp(i)
                    for i in range(2, npair):
                        emit_av(i - 2)
                        if i == 2 and pending_epi[0] is not None:
                            pending_epi[0]()
                            pending_epi[0] = None
                        emit_qk_exp(i)
                    for i in range(max(0, npair - 2), npair):
                        emit_av(i)

                    # -- epilogue part 1: drain oT so the next chunk starts
                    oc = ocpool.tile([DA, QCHUNK], bf16, tag="oc", name="oc")
                    nc.vector.tensor_copy(oc[:, 0:qw], oT[:, 0:qw])
                    srow = stage.tile([1, QCHUNK], f32, tag="srow")
                    nc.vector.tensor_copy(srow[0:1, 0:qw], oT[64:65, 0:qw])

                    # cast the rowsum row to bf16 so the per-128-block
                    # transpose matmuls are single cheap bf16 MMs instead of
                    # fp32 LOW_HIGH pairs; reciprocal happens after the
                    # transpose on a tiny [128, nqt] tile (reciprocal cost
                    # scales with free size).
                    srow_bf = stage.tile([1, QCHUNK], bf16, tag="srowbf")
                    nc.vector.tensor_copy(srow_bf[0:1, 0:qw], srow[0:1, 0:qw])

                    def epilogue(qo=qo, qw=qw, oc=oc, srow_bf=srow_bf):
                        nqt = -(-qw // 128)
                        rps = mps.tile([128, 8], f32, tag="sm", name="rps")
                        for t in range(nqt):
                            rem = min(128, qw - t * 128)
                            nc.tensor.matmul(rps[0:rem, t:t + 1],
                                             srow_bf[0:1, t * 128:t * 128 + rem],
                                             ones1[0:1, 0:1],
                                             start=True, stop=True)
                        recip = stage.tile([128, 8], f32, tag="recip")
                        nc.vector.reciprocal(recip[:, 0:nqt], rps[:, 0:nqt])
                        for t in range(nqt):
                            rem = min(128, qw - t * 128)
                            pps2 = mps.tile([128, 320], f32, tag="sm",
                                            name="pps2")
                            nc.tensor.matmul(pps2[0:rem, :],
                                             oc[0:64, t * 128:t * 128 + rem],
                                             wo_r[:], start=True, stop=True)
                            ot_sb = outsb.tile([128, 320], f32, tag="osb")
                            nc.vector.tensor_scalar_mul(ot_sb[0:rem, :],
                                                        pps2[0:rem, :],
                                                        recip[0:rem, t:t + 1])
                            nc.sync.dma_start(
                                out_d[qo + t * 128:qo + t * 128 + rem, :],
                                ot_sb[0:rem, :])

                    if pending_epi[0] is not None:
                        pending_epi[0]()
                    pending_epi[0] = epilogue
                if pending_epi[0] is not None:
                    pending_epi[0]()
                    pending_epi[0] = None

    nc.compile()
    return nc


def _get_compiled(n0=N, m0=M):
    key = (n0, m0)
    if key not in _compiled:
        _compiled[key] = _build_program(n0=n0, m0=m0)
    return _compiled[key]


def kernel(x, context, mask1, mask2, Wq, Wk, Wv, Wo, bo):
    from concourse import bass_utils
    import ml_dtypes

    global _last_in_maps, _last_key

    bf16 = ml_dtypes.bfloat16
    x = np.asarray(x, dtype=np.float32)
    context = np.asarray(context, dtype=np.float32)
    mask1 = np.asarray(mask1, dtype=np.float32)
    mask2 = np.asarray(mask2, dtype=np.float32)
    Wq = np.asarray(Wq, dtype=np.float32)
    Wk = np.asarray(Wk, dtype=np.float32)
    Wv = np.asarray(Wv, dtype=np.float32)
    Wo = np.asarray(Wo, dtype=np.float32)
    bo = np.asarray(bo, dtype=np.float32)

    b = x.shape[0]
    assert b == 1 and x.shape[1] == N and context.shape[1] == M

    # nearest-resize masks exactly as the reference does
    dxq = int((N // 12) ** 0.5)
    mH, mW = 4 * dxq, 3 * dxq
    dxk = int((M // 12) ** 0.5)
    mh, mw = 4 * dxk, 3 * dxk
    Hm, Wm = mask1.shape[-2], mask1.shape[-1]
    m1 = mask1[0, 0][(np.arange(mH) * Hm) // mH][:, (np.arange(mW) * Wm) // mW] >= 0.5
    m2 = mask2[0, 0][(np.arange(mh) * Hm) // mh][:, (np.arange(mw) * Wm) // mw] >= 0.5

    m1f = m1.reshape(-1)
    m2f = m2.reshape(-1)

    # group unmasked rows/cols first -> every q chunk is pure, and masked
    # chunks use a short k loop
    qperm = np.argsort(m1f, kind="stable")       # False (unmasked) first
    kperm = np.argsort(m2f, kind="stable")
    n0 = int((~m1f).sum())
    m0 = int((~m2f).sum())
    if n0 == N or m0 == 0:
        # no masked q rows (or degenerate mask): single dense region
        qperm = np.arange(N)
        kperm = np.arange(M)
        n0, m0 = N, M

    m2neg = np.where(m2f[kperm], np.float32(NEG), np.float32(0.0))
    xT = np.ascontiguousarray(x[0].T[:, qperm]).astype(bf16)
    ctxT = np.ascontiguousarray(context[0].T[:, kperm]).astype(bf16)

    def pack3(w):
        # [320, 64] -> [128, 192] (c-tiles of 128/128/64 side by side)
        p = np.zeros((128, 192), np.float32)
        p[:, 0:64] = w[0:128]
        p[:, 64:128] = w[128:256]
        p[0:64, 128:192] = w[256:320]
        return p

    def wpack(h):
        p = np.zeros((128, 896), np.float32)
        p[:, 0:192] = pack3(Wq[:, h * D:(h + 1) * D] * np.float32(SCALE))
        p[:, 192:384] = pack3(Wk[:, h * D:(h + 1) * D])
        p[:, 384:576] = pack3(Wv[:, h * D:(h + 1) * D])
        p[0:64, 576:896] = Wo[h * D:(h + 1) * D, :]
        return p.astype(bf16)

    m2n_tiles = m2neg.reshape(NKT, 128).T.copy()  # [128, NKT] fp32
    eye = np.eye(64, dtype=np.float32)

    in_maps = []
    for h in range(HEADS):
        in_maps.append({
            "xt": xT,
            "ctxt": ctxT,
            "wpack": wpack(h),
            "eye": eye,
            "m2neg": m2n_tiles,
        })
    _last_in_maps = in_maps
    _last_key = (n0, m0)

    nc = _get_compiled(n0, m0)
    res = bass_utils.run_bass_kernel_spmd(nc, in_maps, list(range(HEADS)))
    out = np.zeros((N, C), dtype=np.float32)
    for h in range(HEADS):
        out += res.results[h]["out"]
    out += bo
    inv = np.empty(N, dtype=np.int64)
    inv[qperm] = np.arange(N)
    out = out[inv]
    return out.reshape(1, N, C)


# revision 28
# speedup vs baseline: 1.2611x; 1.0313x over previous
"""Trainium2 Bass kernel for nn_CrossAttention_43258910605402.

Masked cross-attention, head-parallel over 8 NeuronCores (one head per core).

Math (per head h):
  q = x @ Wq[:, 64h:64h+64] * d^-0.5          [n=6912, 64]
  k = ctx @ Wk[:, 64h:64h+64]                 [m=3072, 64]
  v = ctx @ Wv[:, 64h:64h+64]                 [m=3072, 64]
  S = q @ k^T  (masked entries -> -inf)       [n, m]
  A = exp(S)   (no row-max: |S| <= ~10 here; masked -> exp = 0)
  out_h = (A @ v) / rowsum(A)                 [n, 64]
  partial = out_h @ Wo[64h:64h+64, :]         [n, 320]
Host: out = sum_h partial_h + bo.

The mask is rank-1 (m1_i & m2_j).  Host permutes q rows and k cols so
unmasked entries come first (n0 / m0 split).  Chunks of q are then PURE:
unmasked chunks attend all of k with no mask at all; masked chunks attend
only the first ceil(m0/128) k-tiles, and only the last (partial) k-tile
needs masking, applied as a per-partition bias (-1e30) on the exp
activation -- so the contraction stays exactly 64.

With K=64 the QK^T matmuls run 2-way row-tiled: even k-tiles occupy PE
rows 0-63 (stationary at SBUF partitions 0-63), odd k-tiles rows 64-127,
executing concurrently into the two halves of one [128, 1024] PSUM pair.
One exp ACTIVATE covers the pair.  A@V uses a [128, 65] stationary
(v plus a ones column that yields rowsum(A) as output row 64);
normalization is deferred into a per-partition scalar multiply after the
output projection.  All matmul operands are bf16 (fp32 PSUM accumulate),
which also halves input DMA.
"""

import numpy as np

HEADS = 8
D = 64
DA = 65          # d + 1 ones row for fused rowsum
N = 6912         # query positions
M = 3072         # key positions
C = 320          # model dim
SCALE = D ** -0.5
NEG = -1e30
NKT = M // 128

_compiled = {}
_last_in_maps = None
_last_key = None


def _chunks(total, size, base=0):
    out = []
    o = 0
    while o < total:
        w = min(size, total - o)
        out.append((base + o, w))
        o += w
    return out


def _build_program(n0=N, m0=M, QCHUNK=512):
    # n0: q rows 0..n0-1 are unmasked (attend all k); rows n0.. attend only
    # k < m0.  NKT_SHORT = ceil(m0/128) k-tiles, with a -1e30 per-partition
    # exp bias killing the masked tail of the last short tile.
    import concourse.bacc as bacc
    import concourse.tile as tile
    import concourse.mybir as mybir

    NKT_SHORT = max(1, -(-m0 // 128))
    bias_tile = NKT_SHORT - 1 if (m0 % 128 != 0 and n0 < N) else None
    f32 = mybir.dt.float32
    f32r = mybir.dt.float32r
    bf16 = mybir.dt.bfloat16
    EXP = mybir.ActivationFunctionType.Exp

    nc = bacc.Bacc("TRN2", target_bir_lowering=False, debug=False)

    xt_d = nc.dram_tensor("xt", [C, N], bf16, kind="ExternalInput").ap()
    ctxt_d = nc.dram_tensor("ctxt", [C, M], bf16, kind="ExternalInput").ap()
    # packed weights: [128, 896] bf16 = wq(192) wk(192) wv(192) | wo 64x320
    wp_d = nc.dram_tensor("wpack", [128, 896], bf16, kind="ExternalInput").ap()
    eye_d = nc.dram_tensor("eye", [64, 64], f32r, kind="ExternalInput").ap()
    m2n_d = nc.dram_tensor("m2neg", [128, NKT], f32, kind="ExternalInput").ap()
    out_d = nc.dram_tensor("out", [N, C], f32, kind="ExternalOutput").ap()

    CCH = [(0, 128), (128, 128), (256, 64)]   # contraction tiles over C=320

    with tile.TileContext(nc) as tc:
        with (
            tc.tile_pool(name="persist", bufs=1) as persist,
            tc.tile_pool(name="stage", bufs=3) as stage,
            tc.tile_pool(name="qpool", bufs=2) as qpool,
            tc.tile_pool(name="attn", bufs=4) as apool,
            tc.tile_pool(name="oc", bufs=2) as ocpool,
            tc.tile_pool(name="outsb", bufs=3) as outsb,
        ):
            # ---- constants / weights -------------------------------------
            wp_r = persist.tile([128, 896], bf16, tag="wpack")
            nc.sync.dma_start(wp_r[:], wp_d[:])
            eye = persist.tile([64, 64], f32r, tag="eye")
            nc.sync.dma_start(eye[:], eye_d[:])
            m2nt = persist.tile([128, NKT], f32, tag="m2nt")
            nc.sync.dma_start(m2nt[:], m2n_d[:])
            ones1 = persist.tile([1, 1], bf16, tag="ones1")
            nc.vector.memset(ones1[:], 1.0)
            # dummy exp so the ~2.7us ACT table load overlaps the prep phase
            warm = persist.tile([1, 8], f32, tag="actwarm")
            nc.vector.memset(warm[:], 0.0)
            nc.scalar.activation(warm[:], warm[:], EXP)
            wq_r = wp_r[:, 0:192]
            wk_r = wp_r[:, 192:384]
            wv_r = wp_r[:, 384:576]
            wo_r = wp_r[0:64, 576:896]

            def wslice(wr, i):
                c0, cw = CCH[i]
                return wr[0:cw, i * 64:(i + 1) * 64]

            # ---- ctx^T / x^T (direct DMA, host-transposed, bf16) ---------
            ct = [persist.tile([128, M], bf16, tag="ct0", name="ct0"),
                  persist.tile([128, M], bf16, tag="ct1", name="ct1"),
                  persist.tile([64, M], bf16, tag="ct2", name="ct2")]

            # k row-packed for 2-way row tiling: partitions 0-63 hold d-dims
            # of even k-tiles, 64-127 of odd k-tiles; tile j's stationary is
            # kt2[64*(j%2):+64, (j//2)*128:+128].
            kt2 = persist.tile([128, M // 2], bf16, tag="kt2")
            vt = persist.tile([64, M], f32r, tag="vt")
            vaug = persist.tile([128, NKT, DA], bf16, tag="vaug")
            ones_col = persist.tile([128, NKT, 1], bf16, tag="ones_col")
            nc.vector.memset(ones_col[:], 1.0)
            nc.vector.tensor_copy(vaug[:, :, 64:65], ones_col[:])
            # q duplicated into both partition halves (moving operand for
            # row-tiled even/odd QK matmuls)
            qt2 = persist.tile([128, N], bf16, tag="qt2")
            assert QCHUNK == 512
            with (
                tc.tile_pool(name="sps", bufs=2, space="PSUM") as sps,
                tc.tile_pool(name="ops", bufs=2, space="PSUM") as ops,
                tc.tile_pool(name="mps", bufs=2, space="PSUM") as mps,
            ):
                kv_chunks = _chunks(M, 512)
                kv_next = [0]

                def emit_kv():
                    o, w = kv_chunks[kv_next[0]]
                    kv_next[0] += 1
                    for i, (c0, cw) in enumerate(CCH):
                        nc.gpsimd.dma_start(ct[i][0:cw, o:o + w],
                                            ctxt_d[c0:c0 + cw, o:o + w])
                    kps = mps.tile([64, 512], f32, tag="sm", name="kps")
                    vps = mps.tile([64, 512], f32, tag="sm", name="vps")
                    for i in range(3):
                        nc.tensor.matmul(kps[:, 0:w], wslice(wk_r, i),
                                         ct[i][0:CCH[i][1], o:o + w],
                                         start=(i == 0), stop=(i == 2))
                        nc.tensor.matmul(vps[:, 0:w], wslice(wv_r, i),
                                         ct[i][0:CCH[i][1], o:o + w],
                                         start=(i == 0), stop=(i == 2))
                    # scatter k 128-blocks into the row-packed layout.
                    # DVE copies are lane-wise (cannot shift partition base),
                    # so cast once to SBUF then move blocks with SBUF->SBUF
                    # DMA, which can place odd tiles at partitions 64-127.
                    ktmp = stage.tile([64, 512], bf16, tag="ktmp", bufs=2)
                    nc.vector.tensor_copy(ktmp[:, 0:w], kps[:, 0:w])
                    for b in range(w // 128):
                        j = o // 128 + b          # absolute k-tile index
                        half = 64 * (j % 2)
                        nc.sync.dma_start(
                            kt2[half:half + 64,
                                (j // 2) * 128:(j // 2) * 128 + 128],
                            ktmp[:, b * 128:(b + 1) * 128])
                    nc.vector.tensor_copy(vt[:, o:o + w], vps[:, 0:w])
                    for j in range(o // 128, (o + w) // 128):
                        vp = mps.tile([128, 64], f32r, tag="sm", name="vp")
                        nc.tensor.transpose(vp[:], vt[:, j * 128:(j + 1) * 128],
                                            eye[:])
                        nc.vector.tensor_copy(vaug[:, j, 0:64], vp[:])

                qprep_chunks = _chunks(N, 512)
                qprep_next = [0]

                def emit_qprep():
                    qo, qw = qprep_chunks[qprep_next[0]]
                    qprep_next[0] += 1
                    xt = [qpool.tile([128, 512], bf16, tag="xt0", name="xt0"),
                          qpool.tile([128, 512], bf16, tag="xt1", name="xt1"),
                          qpool.tile([64, 512], bf16, tag="xt2", name="xt2")]
                    for i, (c0, cw) in enumerate(CCH):
                        nc.gpsimd.dma_start(xt[i][0:cw, 0:qw],
                                            xt_d[c0:c0 + cw, qo:qo + qw])
                    qp = mps.tile([64, 512], f32, tag="sm", name="qp")
                    for i in range(3):
                        nc.tensor.matmul(qp[0:64, 0:qw], wslice(wq_r, i),
                                         xt[i][0:CCH[i][1], 0:qw],
                                         start=(i == 0), stop=(i == 2))
                    nc.vector.tensor_copy(qt2[0:64, qo:qo + qw], qp[0:64, 0:qw])
                    # duplicate into partitions 64-127 (moving operand for the
                    # odd row-tile) -- cross-partition, so SBUF->SBUF DMA
                    nc.sync.dma_start(qt2[64:128, qo:qo + qw],
                                      qt2[0:64, qo:qo + qw])

                pending_epi = [None]
                chunk_list = _chunks(n0, QCHUNK) + _chunks(N - n0, QCHUNK, n0)
                for (qo, qw) in chunk_list:
                    # keep q-prep one main-chunk ahead of consumption
                    target = min(N, qo + qw + QCHUNK)
                    while (qprep_next[0] < len(qprep_chunks)
                           and qprep_chunks[qprep_next[0]][0] < target):
                        emit_qprep()
                    masked = qo >= n0
                    nkt_c = NKT_SHORT if masked else NKT

                    # -- attention over k-tile pairs -----------------------
                    # The AV of pair i is emitted together with the QK of
                    # pair i+2 (software pipeline), so the PE never waits on
                    # the exp of the pair it just computed.  oT is
                    # double-buffered across chunks, so the first AV of a
                    # chunk starts while the previous chunk's oT drains.
                    oT = ops.tile([DA, QCHUNK], f32, tag="oT")
                    pair_list = []
                    jj = 0
                    while jj < nkt_c:
                        pair_list.append((jj, min(2, nkt_c - jj)))
                        jj += pair_list[-1][1]
                    at_tiles = {}

                    def emit_qk_exp(i):
                        jj, pair = pair_list[i]
                        while (kv_next[0] < len(kv_chunks)
                               and kv_next[0] * 4 < min(nkt_c, jj + 8)):
                            emit_kv()
                        s_ps = sps.tile([128, 1024], f32, tag="s")
                        for p in range(pair):
                            j = jj + p
                            half = 64 * (j % 2)
                            nc.tensor.matmul(
                                s_ps[:, p * 512:p * 512 + qw],
                                kt2[half:half + 64,
                                    (j // 2) * 128:(j // 2) * 128 + 128],
                                qt2[half:half + 64, qo:qo + qw],
                                start=True, stop=True)
                        at = apool.tile([128, 1024], bf16, tag="attn")
                        needs_bias = (masked and bias_tile is not None
                                      and jj <= bias_tile < jj + pair)
                        if pair == 2 and qw == 512 and not needs_bias:
                            nc.scalar.activation(at[:, 0:1024], s_ps[:, 0:1024],
                                                 EXP)
                        else:
                            for p in range(pair):
                                j = jj + p
                                if masked and j == bias_tile:
                                    nc.scalar.activation(
                                        at[:, p * 512:p * 512 + qw],
                                        s_ps[:, p * 512:p * 512 + qw], EXP,
                                        bias=m2nt[:, j:j + 1])
                                else:
                                    nc.scalar.activation(
                                        at[:, p * 512:p * 512 + qw],
                                        s_ps[:, p * 512:p * 512 + qw], EXP)
                        at_tiles[i] = at

                    def emit_av(i):
                        jj, pair = pair_list[i]
                        at = at_tiles.pop(i)
                        for p in range(pair):
                            j = jj + p
                            nc.tensor.matmul(oT[:, 0:qw], vaug[:, j, :],
                                             at[:, p * 512:p * 512 + qw],
                                             start=(j == 0),
                                             stop=(j == nkt_c - 1))

                    # two pairs of QK (64-row mode) then two AVs (128-row
                    # mode) per super-iteration -- halves PE mode-switch
                    # drains vs alternating every pair
                    npair = len(pair_list)
                    for i in range(min(2, npair)):
                        emit_qk_exp(i)
                    av_next = 0
                    i = 2
                    first_steady = True
                    while i < npair:
                        take = min(2, npair - i)
                        for t in range(take):
                            emit_qk_exp(i + t)
                        if first_steady and pending_epi[0] is not None:
                            pending_epi[0]()
                            pending_epi[0] = None
                        first_steady = False
                        for t in range(take):
                            emit_av(av_next)
                            av_next += 1
                        i += take
                    while av_next < npair:
                        emit_av(av_next)
                        av_next += 1

                    # -- epilogue part 1: drain oT so the next chunk starts
                    oc = ocpool.tile([DA, QCHUNK], bf16, tag="oc", name="oc")
                    nc.vector.tensor_copy(oc[:, 0:qw], oT[:, 0:qw])
                    srow = stage.tile([1, QCHUNK], f32, tag="srow")
                    nc.vector.tensor_copy(srow[0:1, 0:qw], oT[64:65, 0:qw])

                    # cast the rowsum row to bf16 so the per-128-block
                    # transpose matmuls are single cheap bf16 MMs instead of
                    # fp32 LOW_HIGH pairs; reciprocal happens after the
                    # transpose on a tiny [128, nqt] tile (reciprocal cost
                    # scales with free size).
                    srow_bf = stage.tile([1, QCHUNK], bf16, tag="srowbf")
                    nc.vector.tensor_copy(srow_bf[0:1, 0:qw], srow[0:1, 0:qw])

                    def epilogue(qo=qo, qw=qw, oc=oc, srow_bf=srow_bf):
                        nqt = -(-qw // 128)
                        rps = mps.tile([128, 8], f32, tag="sm", name="rps")
                        for t in range(nqt):
                            rem = min(128, qw - t * 128)
                            nc.tensor.matmul(rps[0:rem, t:t + 1],
                                             srow_bf[0:1, t * 128:t * 128 + rem],
                                             ones1[0:1, 0:1],
                                             start=True, stop=True)
                        recip = stage.tile([128, 8], f32, tag="recip")
                        nc.vector.reciprocal(recip[:, 0:nqt], rps[:, 0:nqt])
                        for t in range(nqt):
                            rem = min(128, qw - t * 128)
                            pps2 = mps.tile([128, 320], f32, tag="sm",
                                            name="pps2")
                            nc.tensor.matmul(pps2[0:rem, :],
                                             oc[0:64, t * 128:t * 128 + rem],
                                             wo_r[:], start=True, stop=True)
                            ot_sb = outsb.tile([128, 320], f32, tag="osb")
                            nc.vector.tensor_scalar_mul(ot_sb[0:rem, :],
                                                        pps2[0:rem, :],
                                                        recip[0:rem, t:t + 1])
                            nc.sync.dma_start(
                                out_d[qo + t * 128:qo + t * 128 + rem, :],
                                ot_sb[0:rem, :])

                    if pending_epi[0] is not None:
                        pending_epi[0]()
                    pending_epi[0] = epilogue
                if pending_epi[0] is not None:
                    pending_epi[0]()
                    pending_epi[0] = None

    nc.compile()
    return nc


def _get_compiled(n0=N, m0=M):
    key = (n0, m0)
    if key not in _compiled:
        _compiled[key] = _build_program(n0=n0, m0=m0)
    return _compiled[key]


def kernel(x, context, mask1, mask2, Wq, Wk, Wv, Wo, bo):
    from concourse import bass_utils
    import ml_dtypes

    global _last_in_maps, _last_key

    bf16 = ml_dtypes.bfloat16
    x = np.asarray(x, dtype=np.float32)
    context = np.asarray(context, dtype=np.float32)
    mask1 = np.asarray(mask1, dtype=np.float32)
    mask2 = np.asarray(mask2, dtype=np.float32)
    Wq = np.asarray(Wq, dtype=np.float32)
    Wk = np.asarray(Wk, dtype=np.float32)
    Wv = np.asarray(Wv, dtype=np.float32)
    Wo = np.asarray(Wo, dtype=np.float32)
    bo = np.asarray(bo, dtype=np.float32)

    b = x.shape[0]
    assert b == 1 and x.shape[1] == N and context.shape[1] == M

    # nearest-resize masks exactly as the reference does
    dxq = int((N // 12) ** 0.5)
    mH, mW = 4 * dxq, 3 * dxq
    dxk = int((M // 12) ** 0.5)
    mh, mw = 4 * dxk, 3 * dxk
    Hm, Wm = mask1.shape[-2], mask1.shape[-1]
    m1 = mask1[0, 0][(np.arange(mH) * Hm) // mH][:, (np.arange(mW) * Wm) // mW] >= 0.5
    m2 = mask2[0, 0][(np.arange(mh) * Hm) // mh][:, (np.arange(mw) * Wm) // mw] >= 0.5

    m1f = m1.reshape(-1)
    m2f = m2.reshape(-1)

    # group unmasked rows/cols first -> every q chunk is pure, and masked
    # chunks use a short k loop
    qperm = np.argsort(m1f, kind="stable")       # False (unmasked) first
    kperm = np.argsort(m2f, kind="stable")
    n0 = int((~m1f).sum())
    m0 = int((~m2f).sum())
    if n0 == N or m0 == 0:
        # no masked q rows (or degenerate mask): single dense region
        qperm = np.arange(N)
        kperm = np.arange(M)
        n0, m0 = N, M

    m2neg = np.where(m2f[kperm], np.float32(NEG), np.float32(0.0))
    xT = np.ascontiguousarray(x[0].T[:, qperm]).astype(bf16)
    ctxT = np.ascontiguousarray(context[0].T[:, kperm]).astype(bf16)

    def pack3(w):
        # [320, 64] -> [128, 192] (c-tiles of 128/128/64 side by side)
        p = np.zeros((128, 192), np.float32)
        p[:, 0:64] = w[0:128]
        p[:, 64:128] = w[128:256]
        p[0:64, 128:192] = w[256:320]
        return p

    def wpack(h):
        p = np.zeros((128, 896), np.float32)
        p[:, 0:192] = pack3(Wq[:, h * D:(h + 1) * D] * np.float32(SCALE))
        p[:, 192:384] = pack3(Wk[:, h * D:(h + 1) * D])
        p[:, 384:576] = pack3(Wv[:, h * D:(h + 1) * D])
        p[0:64, 576:896] = Wo[h * D:(h + 1) * D, :]
        return p.astype(bf16)

    m2n_tiles = m2neg.reshape(NKT, 128).T.copy()  # [128, NKT] fp32
    eye = np.eye(64, dtype=np.float32)

    in_maps = []
    for h in range(HEADS):
        in_maps.append({
            "xt": xT,
            "ctxt": ctxT,
            "wpack": wpack(h),
            "eye": eye,
            "m2neg": m2n_tiles,
        })
    _last_in_maps = in_maps
    _last_key = (n0, m0)

    nc = _get_compiled(n0, m0)
    res = bass_utils.run_bass_kernel_spmd(nc, in_maps, list(range(HEADS)))
    out = np.zeros((N, C), dtype=np.float32)
    for h in range(HEADS):
        out += res.results[h]["out"]
    out += bo
    inv = np.empty(N, dtype=np.int64)
    inv[qperm] = np.arange(N)
    out = out[inv]
    return out.reshape(1, N, C)
